# revision 12
# baseline (speedup 1.0000x reference)
"""Trainium2 Bass kernel for nn_DWAttEncoder (depth-wise attention encoder).

Strategy (8 NeuronCores, data-parallel over batch B=2048 -> 256 rows/core):
  - Device (per core, per layer l of 33):
      h   = gelu(x[:,l,:] @ W1[l])        bf16 matmul, fp32 PSUM accum
      h   = LN(h)                          fp32 stats, fused scale/bias apply
      hT  = transpose(h)                   PE-transpose (needed as mm2 lhsT)
      v   = hT.T @ W2[l]                   bf16 matmul
      acc += attn[:,l] * LN(v)             attn folded into LN2 affine
  - Host (fp32, ~1.5% of FLOPs, softmax-logit precision critical):
      keys/query path, softmax -> attn;  final residual z_L + gathered acc.
  - b1/b2 are zeros and ln*_g/ln*_b are ones/zeros per the problem spec;
    verified at runtime, with a full-precision host fallback if ever not.
"""

import numpy as np
import ml_dtypes

import concourse.bacc as bacc
import concourse.tile as tile
from concourse import mybir
from concourse.bass_utils import run_bass_kernel_spmd

BF16_NP = ml_dtypes.bfloat16
BF16 = mybir.dt.bfloat16
F32 = mybir.dt.float32
AF = mybir.ActivationFunctionType
OP = mybir.AluOpType

L, D, DB = 33, 2048, 1024
B = 2048
NCORES = 8
BC = B // NCORES      # 256 batch rows per core
MB = BC // 128        # 2 b-tiles of 128 partitions
KD = D // 128         # 16 k-tiles for mm1 contraction
KB = DB // 128        # 8 k-tiles for mm2 contraction
ND1 = DB // 512       # 2 psum chunks for h
ND2 = D // 512        # 4 psum chunks for v
EPS = 1e-5

_cached_nc = None
_last_results = None


def _build():
    global _cached_nc
    if _cached_nc is not None:
        return _cached_nc

    nc = bacc.Bacc("TRN2", target_bir_lowering=False, debug=False,
                   num_devices=NCORES)
    xt = nc.dram_tensor("xt", [L, KD, 128, BC], BF16, kind="ExternalInput")
    w1 = nc.dram_tensor("w1", [L, KD, 128, DB], BF16, kind="ExternalInput")
    w2 = nc.dram_tensor("w2", [L, KB, 128, D], BF16, kind="ExternalInput")
    attn = nc.dram_tensor("attn", [MB, 128, L], F32, kind="ExternalInput")
    out = nc.dram_tensor("out", [MB, 128, D], F32, kind="ExternalOutput")

    with tile.TileContext(nc) as tc:
        with (
            tc.tile_pool(name="const", bufs=1) as cpool,
            tc.tile_pool(name="w1p", bufs=2) as w1p,
            tc.tile_pool(name="w2p", bufs=2) as w2p,
            tc.tile_pool(name="xtp", bufs=2) as xtp,
            tc.tile_pool(name="hp", bufs=2) as hp,
            tc.tile_pool(name="htp", bufs=2) as htp,
            tc.tile_pool(name="tmpp", bufs=4) as tmpp,
            tc.tile_pool(name="stats", bufs=6) as stats,
            tc.tile_pool(name="ph", bufs=2, space="PSUM") as php,
            tc.tile_pool(name="pv", bufs=6, space="PSUM") as pvp,
        ):
            def emit_loads(l, n_split):
                xt_sb = xtp.tile([128, KD, BC], BF16, tag="xt")
                s1 = KD // min(n_split, 4)
                for s in range(min(n_split, 4)):
                    nc.sync.dma_start(
                        out=xt_sb[:, s * s1:(s + 1) * s1, :],
                        in_=xt[l, s * s1:(s + 1) * s1].rearrange(
                            "k p b -> p k b"))
                w1_sb = w1p.tile([128, KD, DB], BF16, tag="w1")
                s2 = KD // n_split
                for s in range(n_split):
                    nc.sync.dma_start(
                        out=w1_sb[:, s * s2:(s + 1) * s2, :],
                        in_=w1[l, s * s2:(s + 1) * s2].rearrange(
                            "k p n -> p k n"))
                w2_sb = w2p.tile([128, KB, D], BF16, tag="w2")
                s3 = KB // n_split if KB >= n_split else 1
                for s in range(KB // s3):
                    nc.sync.dma_start(
                        out=w2_sb[:, s * s3:(s + 1) * s3, :],
                        in_=w2[l, s * s3:(s + 1) * s3].rearrange(
                            "k p n -> p k n"))
                return xt_sb, w1_sb, w2_sb

            # layer-0 inputs first, finely split, so the first matmul can
            # start as early as possible
            pending = emit_loads(0, 8)

            eps_t = cpool.tile([128, 1], F32)
            nc.vector.memset(eps_t, EPS)
            attn_sb = cpool.tile([128, MB, L], F32)
            nc.sync.dma_start(out=attn_sb, in_=attn[:].rearrange("m p l -> p m l"))
            acc = cpool.tile([128, MB, D], F32)

            for l in range(L):
                xt_sb, w1_sb, w2_sb = pending if l == 0 else emit_loads(l, 8 if l == 1 else 4)

                # ---- phase A per b-tile: mm1 + gelu + LN1 ----
                h_ln = []
                for m in range(MB):
                    msl = slice(m * 128, (m + 1) * 128)
                    phs = [php.tile([128, 512], F32, tag="ph", name=f"ph{n}")
                           for n in range(ND1)]
                    for k in range(KD):
                        for n in range(ND1):
                            nc.tensor.matmul(
                                phs[n], lhsT=xt_sb[:, k, msl],
                                rhs=w1_sb[:, k, n * 512:(n + 1) * 512],
                                start=(k == 0), stop=(k == KD - 1))
                    hg = hp.tile([128, DB], BF16, tag="hg")
                    for n in range(ND1):
                        nc.scalar.activation(
                            out=hg[:, n * 512:(n + 1) * 512], in_=phs[n],
                            func=AF.Gelu)
                    st1 = stats.tile([128, ND1, 6], F32, tag="st1")
                    for n in range(ND1):
                        nc.vector.bn_stats(
                            out=st1[:, n, :], in_=hg[:, n * 512:(n + 1) * 512])
                    mv1 = stats.tile([128, 2], F32, tag="mv1")
                    nc.vector.bn_aggr(out=mv1, in_=st1)
                    rs1 = stats.tile([128, 1], F32, tag="rs1")
                    nc.scalar.activation(out=rs1, in_=mv1[:, 1:2], func=AF.Sqrt,
                                         bias=eps_t)
                    nc.vector.reciprocal(out=rs1, in_=rs1)
                    c1 = stats.tile([128, 1], F32, tag="c1")
                    nc.vector.tensor_scalar(out=c1, in0=mv1[:, 0:1], scalar1=rs1,
                                            scalar2=-1.0, op0=OP.mult,
                                            op1=OP.mult)
                    hl = hp.tile([128, DB], BF16, tag="hl")
                    nc.scalar.activation(out=hl, in_=hg, func=AF.Identity,
                                         bias=c1, scale=rs1)
                    h_ln.append(hl)

                # ---- phase B per b-tile: transpose + mm2 + LN2 + accumulate ----
                for m in range(MB):
                    hl = h_ln[m]
                    # h_ln -> hT via the DMA xbar transpose engine (off PE);
                    # 8 chunked transfers so they spread across queues and
                    # mm2 (k-outer) can start on chunk 0 immediately
                    ht = htp.tile([128, KB, 128], BF16, tag="ht")
                    for j in range(KB):
                        nc.scalar.dma_start_transpose(
                            ht[:, j, :], hl[:, j * 128:(j + 1) * 128])

                    st2 = stats.tile([128, ND2, 6], F32, tag="st2")
                    pvs = [pvp.tile([128, 512], F32, tag="pv", name=f"pv{n}")
                           for n in range(ND2)]
                    for k in range(KB):
                        for n in range(ND2):
                            nc.tensor.matmul(
                                pvs[n], lhsT=ht[:, k, :],
                                rhs=w2_sb[:, k, n * 512:(n + 1) * 512],
                                start=(k == 0), stop=(k == KB - 1))
                    for n in range(ND2):
                        nc.vector.bn_stats(out=st2[:, n, :], in_=pvs[n])
                    mv2 = stats.tile([128, 2], F32, tag="mv2")
                    nc.vector.bn_aggr(out=mv2, in_=st2)
                    rs2 = stats.tile([128, 1], F32, tag="rs2")
                    nc.scalar.activation(out=rs2, in_=mv2[:, 1:2], func=AF.Sqrt,
                                         bias=eps_t)
                    nc.vector.reciprocal(out=rs2, in_=rs2)
                    a2 = stats.tile([128, 1], F32, tag="a2")
                    nc.vector.tensor_mul(out=a2, in0=rs2,
                                         in1=attn_sb[:, m, l:l + 1])
                    c2 = stats.tile([128, 1], F32, tag="c2")
                    nc.vector.tensor_scalar(out=c2, in0=mv2[:, 0:1], scalar1=a2,
                                            scalar2=-1.0, op0=OP.mult,
                                            op1=OP.mult)
                    for n in range(ND2):
                        nsl = slice(n * 512, (n + 1) * 512)
                        if l == 0:
                            nc.vector.tensor_scalar(
                                out=acc[:, m, nsl], in0=pvs[n], scalar1=a2,
                                scalar2=c2, op0=OP.mult, op1=OP.add)
                        else:
                            tmp = tmpp.tile([128, 512], BF16, tag="tmp")
                            nc.scalar.activation(out=tmp, in_=pvs[n],
                                                 func=AF.Identity, bias=c2,
                                                 scale=a2)
                            nc.vector.tensor_add(out=acc[:, m, nsl],
                                                 in0=acc[:, m, nsl], in1=tmp)

            for m in range(MB):
                for q in range(4):
                    nc.sync.dma_start(
                        out=out[m, :, q * 512:(q + 1) * 512],
                        in_=acc[:, m, q * 512:(q + 1) * 512])

    nc.compile()
    _cached_nc = nc
    return nc


# ---------------- host-side math (fp32) ----------------

def _gelu(x):
    from scipy.special import erf
    return 0.5 * x * (1.0 + erf(x / np.sqrt(2.0, dtype=np.float32)))


def _ln(x, g, b):
    mu = x.mean(-1, keepdims=True, dtype=np.float32)
    var = np.square(x - mu).mean(-1, keepdims=True, dtype=np.float32)
    return (x - mu) / np.sqrt(var + EPS) * g + b


def _elu(x):
    return np.where(x > 0, x, np.expm1(np.minimum(x, 0.0)))


def _host_query_attn(zL, pos_emb, Wk, Wq1, bq1, lnq_g, lnq_b, Wq2, bq2):
    keys = pos_emb @ Wk                                   # [L, D]
    hq = _gelu(zL @ Wq1 + bq1)
    hq = _ln(hq, lnq_g, lnq_b)
    q_tr = hq @ Wq2 + bq2
    query = 1.0 + _elu(zL + q_tr)                         # [B, D]
    s = query @ keys.T                                    # [B, L]
    s -= s.max(-1, keepdims=True)
    e = np.exp(s)
    return e / e.sum(-1, keepdims=True)


def _host_reference(x, pos_emb, Wk, W1, b1, ln1_g, ln1_b, W2, b2, ln2_g,
                    ln2_b, Wq1, bq1, lnq_g, lnq_b, Wq2, bq2):
    """Full-precision fallback (only used if the affine params are ever
    non-trivial, which the problem spec's fills make impossible)."""
    zL = x[:, -1, :]
    attn = _host_query_attn(zL, pos_emb, Wk, Wq1, bq1, lnq_g, lnq_b, Wq2, bq2)
    acc = np.zeros_like(zL)
    for l in range(L):
        h = _gelu(x[:, l, :] @ W1[l] + b1[l])
        h = _ln(h, ln1_g[l], ln1_b[l])
        v = h @ W2[l] + b2[l]
        v = _ln(v, ln2_g[l], ln2_b[l])
        acc += attn[:, l:l + 1] * v
    return zL + acc


def kernel(x, pos_emb, Wk, W1, b1, ln1_g, ln1_b, W2, b2, ln2_g, ln2_b,
           Wq1, bq1, lnq_g, lnq_b, Wq2, bq2):
    global _last_results
    f32 = np.float32
    x = np.asarray(x, f32)
    pos_emb = np.asarray(pos_emb, f32)
    Wk = np.asarray(Wk, f32)
    W1 = np.asarray(W1, f32)
    b1 = np.asarray(b1, f32)
    ln1_g = np.asarray(ln1_g, f32)
    ln1_b = np.asarray(ln1_b, f32)
    W2 = np.asarray(W2, f32)
    b2 = np.asarray(b2, f32)
    ln2_g = np.asarray(ln2_g, f32)
    ln2_b = np.asarray(ln2_b, f32)
    Wq1 = np.asarray(Wq1, f32)
    bq1 = np.asarray(bq1, f32)
    lnq_g = np.asarray(lnq_g, f32)
    lnq_b = np.asarray(lnq_b, f32)
    Wq2 = np.asarray(Wq2, f32)
    bq2 = np.asarray(bq2, f32)

    trivial = (
        not b1.any() and not b2.any()
        and not ln1_b.any() and not ln2_b.any()
        and np.all(ln1_g == 1.0) and np.all(ln2_g == 1.0)
    )
    if not trivial:
        return _host_reference(x, pos_emb, Wk, W1, b1, ln1_g, ln1_b, W2, b2,
                               ln2_g, ln2_b, Wq1, bq1, lnq_g, lnq_b, Wq2, bq2)

    zL = np.ascontiguousarray(x[:, -1, :])
    attn = _host_query_attn(zL, pos_emb, Wk, Wq1, bq1, lnq_g, lnq_b, Wq2, bq2)

    w1b = np.ascontiguousarray(W1.reshape(L, KD, 128, DB)).astype(BF16_NP)
    w2b = np.ascontiguousarray(W2.reshape(L, KB, 128, D)).astype(BF16_NP)

    in_maps = []
    for c in range(NCORES):
        xsb = x[c * BC:(c + 1) * BC].astype(BF16_NP)       # [BC, L, D]
        xt = np.empty((L, KD, 128, BC), BF16_NP)
        for l in range(L):
            xt[l] = xsb[:, l, :].T.reshape(KD, 128, BC)
        attn_c = np.ascontiguousarray(
            attn[c * BC:(c + 1) * BC].reshape(MB, 128, L).astype(f32))
        in_maps.append({"xt": xt, "w1": w1b, "w2": w2b, "attn": attn_c})

    nc = _build()
    res = run_bass_kernel_spmd(nc, in_maps, list(range(NCORES)))
    _last_results = res

    parts = [r["out"].reshape(BC, D).astype(f32) for r in res.results]
    attended = np.concatenate(parts, axis=0)
    return (zL + attended).astype(f32)


# revision 15
# speedup vs baseline: 1.6074x; 1.6074x over previous
"""Trainium2 Bass kernel for nn_DWAttEncoder (depth-wise attention encoder).

Strategy (8 NeuronCores, data-parallel over batch B=2048 -> 256 rows/core):
  - Device (per core, per layer l of 33):
      h   = gelu(x[:,l,:] @ W1[l])        bf16 matmul, fp32 PSUM accum
      h   = LN(h)                          fp32 stats, fused scale/bias apply
      hT  = transpose(h)                   PE-transpose (needed as mm2 lhsT)
      v   = hT.T @ W2[l]                   bf16 matmul
      acc += attn[:,l] * LN(v)             attn folded into LN2 affine
  - Host (fp32, ~1.5% of FLOPs, softmax-logit precision critical):
      keys/query path, softmax -> attn;  final residual z_L + gathered acc.
  - b1/b2 are zeros and ln*_g/ln*_b are ones/zeros per the problem spec;
    verified at runtime, with a full-precision host fallback if ever not.
"""

import numpy as np
import ml_dtypes

import concourse.bacc as bacc
import concourse.tile as tile
from concourse import mybir
from concourse.bass_utils import run_bass_kernel_spmd
from concourse.masks import make_identity

BF16_NP = ml_dtypes.bfloat16
BF16 = mybir.dt.bfloat16
F32 = mybir.dt.float32
AF = mybir.ActivationFunctionType
OP = mybir.AluOpType

L, D, DB = 33, 2048, 1024
B = 2048
NCORES = 8
BC = B // NCORES      # 256 batch rows per core
MB = BC // 128        # 2 b-tiles of 128 partitions
KD = D // 128         # 16 k-tiles for mm1 contraction
KB = DB // 128        # 8 k-tiles for mm2 contraction
ND1 = DB // 512       # 2 psum chunks for h
ND2 = D // 512        # 4 psum chunks for v
EPS = 1e-5

_cached_nc = None
_last_results = None


def _build():
    global _cached_nc
    if _cached_nc is not None:
        return _cached_nc

    nc = bacc.Bacc("TRN2", target_bir_lowering=False, debug=False,
                   num_devices=NCORES)
    xt = nc.dram_tensor("xt", [L, KD, 128, BC], BF16, kind="ExternalInput")
    w1 = nc.dram_tensor("w1", [L, KD, 128, DB], BF16, kind="ExternalInput")
    w2 = nc.dram_tensor("w2", [L, KB, 128, D], BF16, kind="ExternalInput")
    attn = nc.dram_tensor("attn", [MB, 128, L], F32, kind="ExternalInput")
    out = nc.dram_tensor("out", [MB, 128, D], F32, kind="ExternalOutput")

    with tile.TileContext(nc) as tc:
        with (
            tc.tile_pool(name="const", bufs=1) as cpool,
            tc.tile_pool(name="w1p", bufs=2) as w1p,
            tc.tile_pool(name="w2p", bufs=2) as w2p,
            tc.tile_pool(name="xtp", bufs=2) as xtp,
            tc.tile_pool(name="hp", bufs=2) as hp,
            tc.tile_pool(name="htp", bufs=2) as htp,
            tc.tile_pool(name="tmpp", bufs=4) as tmpp,
            tc.tile_pool(name="stats", bufs=6) as stats,
            tc.tile_pool(name="ph", bufs=2, space="PSUM") as php,
            tc.tile_pool(name="pt", bufs=2, space="PSUM") as ptp,
            tc.tile_pool(name="pv", bufs=4, space="PSUM") as pvp,
        ):
            def emit_loads(l, n_split):
                xt_sb = xtp.tile([128, KD, BC], BF16, tag="xt")
                s1 = KD // min(n_split, 4)
                for s in range(min(n_split, 4)):
                    nc.sync.dma_start(
                        out=xt_sb[:, s * s1:(s + 1) * s1, :],
                        in_=xt[l, s * s1:(s + 1) * s1].rearrange(
                            "k p b -> p k b"))
                w1_sb = w1p.tile([128, KD, DB], BF16, tag="w1")
                s2 = KD // n_split
                for s in range(n_split):
                    nc.sync.dma_start(
                        out=w1_sb[:, s * s2:(s + 1) * s2, :],
                        in_=w1[l, s * s2:(s + 1) * s2].rearrange(
                            "k p n -> p k n"))
                w2_sb = w2p.tile([128, KB, D], BF16, tag="w2")
                s3 = KB // n_split if KB >= n_split else 1
                for s in range(KB // s3):
                    nc.sync.dma_start(
                        out=w2_sb[:, s * s3:(s + 1) * s3, :],
                        in_=w2[l, s * s3:(s + 1) * s3].rearrange(
                            "k p n -> p k n"))
                return xt_sb, w1_sb, w2_sb

            # layer-0 inputs first, finely split, so the first matmul can
            # start as early as possible
            pending = emit_loads(0, 8)

            ident = cpool.tile([128, 128], BF16)
            make_identity(nc, ident)
            eps_t = cpool.tile([128, 1], F32)
            nc.vector.memset(eps_t, EPS)
            attn_sb = cpool.tile([128, MB, L], F32)
            nc.sync.dma_start(out=attn_sb, in_=attn[:].rearrange("m p l -> p m l"))
            acc = cpool.tile([128, MB, D], F32)

            for l in range(L):
                xt_sb, w1_sb, w2_sb = pending if l == 0 else emit_loads(l, 8 if l == 1 else 4)

                # ---- phase A per b-tile: mm1 + gelu + LN1 ----
                h_ln = []
                for m in range(MB):
                    msl = slice(m * 128, (m + 1) * 128)
                    phs = [php.tile([128, 512], F32, tag="ph", name=f"ph{n}")
                           for n in range(ND1)]
                    for k in range(KD):
                        for n in range(ND1):
                            nc.tensor.matmul(
                                phs[n], lhsT=xt_sb[:, k, msl],
                                rhs=w1_sb[:, k, n * 512:(n + 1) * 512],
                                start=(k == 0), stop=(k == KD - 1))
                    hg = hp.tile([128, DB], BF16, tag="hg")
                    for n in range(ND1):
                        nc.scalar.activation(
                            out=hg[:, n * 512:(n + 1) * 512], in_=phs[n],
                            func=AF.Gelu)
                    st1 = stats.tile([128, ND1, 6], F32, tag="st1")
                    for n in range(ND1):
                        nc.vector.bn_stats(
                            out=st1[:, n, :], in_=hg[:, n * 512:(n + 1) * 512])
                    mv1 = stats.tile([128, 2], F32, tag="mv1")
                    nc.vector.bn_aggr(out=mv1, in_=st1)
                    rs1 = stats.tile([128, 1], F32, tag="rs1")
                    nc.scalar.activation(out=rs1, in_=mv1[:, 1:2], func=AF.Sqrt,
                                         bias=eps_t)
                    nc.vector.reciprocal(out=rs1, in_=rs1)
                    c1 = stats.tile([128, 1], F32, tag="c1")
                    nc.vector.tensor_scalar(out=c1, in0=mv1[:, 0:1], scalar1=rs1,
                                            scalar2=-1.0, op0=OP.mult,
                                            op1=OP.mult)
                    hl = hp.tile([128, DB], BF16, tag="hl")
                    nc.scalar.activation(out=hl, in_=hg, func=AF.Identity,
                                         bias=c1, scale=rs1)
                    h_ln.append(hl)

                # ---- phase B per b-tile: transpose + mm2 + LN2 + accumulate ----
                for m in range(MB):
                    hl = h_ln[m]
                    ht = htp.tile([128, KB, 128], BF16, tag="ht")
                    for j2 in range(KB // 2):
                        pt = ptp.tile([128, 256], BF16, tag="pt")
                        for jj in range(2):
                            j = j2 * 2 + jj
                            nc.tensor.transpose(
                                pt[:, jj * 128:(jj + 1) * 128],
                                hl[:, j * 128:(j + 1) * 128], ident)
                        nc.vector.tensor_copy(
                            out=ht[:, j2 * 2:j2 * 2 + 2, :],
                            in_=pt[:].rearrange("p (j b) -> p j b", j=2))

                    st2 = stats.tile([128, ND2, 6], F32, tag="st2")
                    pvs = [pvp.tile([128, 512], F32, tag="pv", name=f"pv{n}")
                           for n in range(ND2)]
                    for k in range(KB):
                        for n in range(ND2):
                            nc.tensor.matmul(
                                pvs[n], lhsT=ht[:, k, :],
                                rhs=w2_sb[:, k, n * 512:(n + 1) * 512],
                                start=(k == 0), stop=(k == KB - 1))
                    for n in range(ND2):
                        nc.vector.bn_stats(out=st2[:, n, :], in_=pvs[n])
                    mv2 = stats.tile([128, 2], F32, tag="mv2")
                    nc.vector.bn_aggr(out=mv2, in_=st2)
                    rs2 = stats.tile([128, 1], F32, tag="rs2")
                    nc.scalar.activation(out=rs2, in_=mv2[:, 1:2], func=AF.Sqrt,
                                         bias=eps_t)
                    nc.vector.reciprocal(out=rs2, in_=rs2)
                    a2 = stats.tile([128, 1], F32, tag="a2")
                    nc.vector.tensor_mul(out=a2, in0=rs2,
                                         in1=attn_sb[:, m, l:l + 1])
                    c2 = stats.tile([128, 1], F32, tag="c2")
                    nc.vector.tensor_scalar(out=c2, in0=mv2[:, 0:1], scalar1=a2,
                                            scalar2=-1.0, op0=OP.mult,
                                            op1=OP.mult)
                    for n in range(ND2):
                        nsl = slice(n * 512, (n + 1) * 512)
                        if l == 0:
                            nc.vector.tensor_scalar(
                                out=acc[:, m, nsl], in0=pvs[n], scalar1=a2,
                                scalar2=c2, op0=OP.mult, op1=OP.add)
                        else:
                            tmp = tmpp.tile([128, 512], BF16, tag="tmp")
                            nc.scalar.activation(out=tmp, in_=pvs[n],
                                                 func=AF.Identity, bias=c2,
                                                 scale=a2)
                            nc.vector.tensor_add(out=acc[:, m, nsl],
                                                 in0=acc[:, m, nsl], in1=tmp)

            for m in range(MB):
                for q in range(4):
                    nc.sync.dma_start(
                        out=out[m, :, q * 512:(q + 1) * 512],
                        in_=acc[:, m, q * 512:(q + 1) * 512])

    nc.compile()
    _cached_nc = nc
    return nc


# ---------------- host-side math (fp32) ----------------

def _gelu(x):
    from scipy.special import erf
    return 0.5 * x * (1.0 + erf(x / np.sqrt(2.0, dtype=np.float32)))


def _ln(x, g, b):
    mu = x.mean(-1, keepdims=True, dtype=np.float32)
    var = np.square(x - mu).mean(-1, keepdims=True, dtype=np.float32)
    return (x - mu) / np.sqrt(var + EPS) * g + b


def _elu(x):
    return np.where(x > 0, x, np.expm1(np.minimum(x, 0.0)))


def _host_query_attn(zL, pos_emb, Wk, Wq1, bq1, lnq_g, lnq_b, Wq2, bq2):
    keys = pos_emb @ Wk                                   # [L, D]
    hq = _gelu(zL @ Wq1 + bq1)
    hq = _ln(hq, lnq_g, lnq_b)
    q_tr = hq @ Wq2 + bq2
    query = 1.0 + _elu(zL + q_tr)                         # [B, D]
    s = query @ keys.T                                    # [B, L]
    s -= s.max(-1, keepdims=True)
    e = np.exp(s)
    return e / e.sum(-1, keepdims=True)


def _host_reference(x, pos_emb, Wk, W1, b1, ln1_g, ln1_b, W2, b2, ln2_g,
                    ln2_b, Wq1, bq1, lnq_g, lnq_b, Wq2, bq2):
    """Full-precision fallback (only used if the affine params are ever
    non-trivial, which the problem spec's fills make impossible)."""
    zL = x[:, -1, :]
    attn = _host_query_attn(zL, pos_emb, Wk, Wq1, bq1, lnq_g, lnq_b, Wq2, bq2)
    acc = np.zeros_like(zL)
    for l in range(L):
        h = _gelu(x[:, l, :] @ W1[l] + b1[l])
        h = _ln(h, ln1_g[l], ln1_b[l])
        v = h @ W2[l] + b2[l]
        v = _ln(v, ln2_g[l], ln2_b[l])
        acc += attn[:, l:l + 1] * v
    return zL + acc


def kernel(x, pos_emb, Wk, W1, b1, ln1_g, ln1_b, W2, b2, ln2_g, ln2_b,
           Wq1, bq1, lnq_g, lnq_b, Wq2, bq2):
    global _last_results
    f32 = np.float32
    x = np.asarray(x, f32)
    pos_emb = np.asarray(pos_emb, f32)
    Wk = np.asarray(Wk, f32)
    W1 = np.asarray(W1, f32)
    b1 = np.asarray(b1, f32)
    ln1_g = np.asarray(ln1_g, f32)
    ln1_b = np.asarray(ln1_b, f32)
    W2 = np.asarray(W2, f32)
    b2 = np.asarray(b2, f32)
    ln2_g = np.asarray(ln2_g, f32)
    ln2_b = np.asarray(ln2_b, f32)
    Wq1 = np.asarray(Wq1, f32)
    bq1 = np.asarray(bq1, f32)
    lnq_g = np.asarray(lnq_g, f32)
    lnq_b = np.asarray(lnq_b, f32)
    Wq2 = np.asarray(Wq2, f32)
    bq2 = np.asarray(bq2, f32)

    trivial = (
        not b1.any() and not b2.any()
        and not ln1_b.any() and not ln2_b.any()
        and np.all(ln1_g == 1.0) and np.all(ln2_g == 1.0)
    )
    if not trivial:
        return _host_reference(x, pos_emb, Wk, W1, b1, ln1_g, ln1_b, W2, b2,
                               ln2_g, ln2_b, Wq1, bq1, lnq_g, lnq_b, Wq2, bq2)

    zL = np.ascontiguousarray(x[:, -1, :])
    attn = _host_query_attn(zL, pos_emb, Wk, Wq1, bq1, lnq_g, lnq_b, Wq2, bq2)

    w1b = np.ascontiguousarray(W1.reshape(L, KD, 128, DB)).astype(BF16_NP)
    w2b = np.ascontiguousarray(W2.reshape(L, KB, 128, D)).astype(BF16_NP)

    in_maps = []
    for c in range(NCORES):
        xsb = x[c * BC:(c + 1) * BC].astype(BF16_NP)       # [BC, L, D]
        xt = np.empty((L, KD, 128, BC), BF16_NP)
        for l in range(L):
            xt[l] = xsb[:, l, :].T.reshape(KD, 128, BC)
        attn_c = np.ascontiguousarray(
            attn[c * BC:(c + 1) * BC].reshape(MB, 128, L).astype(f32))
        in_maps.append({"xt": xt, "w1": w1b, "w2": w2b, "attn": attn_c})

    nc = _build()
    res = run_bass_kernel_spmd(nc, in_maps, list(range(NCORES)))
    _last_results = res

    parts = [r["out"].reshape(BC, D).astype(f32) for r in res.results]
    attended = np.concatenate(parts, axis=0)
    return (zL + attended).astype(f32)


# revision 17
# speedup vs baseline: 1.9643x; 1.2220x over previous
"""Trainium2 Bass kernel for nn_DWAttEncoder (depth-wise attention encoder).

Strategy (8 NeuronCores, data-parallel over batch B=2048 -> 256 rows/core):
  - Device (per core, per layer l of 33):
      h   = gelu(x[:,l,:] @ W1[l])        bf16 matmul, fp32 PSUM accum
      h   = LN(h)                          fp32 stats, fused scale/bias apply
      hT  = transpose(h)                   PE-transpose (needed as mm2 lhsT)
      v   = hT.T @ W2[l]                   bf16 matmul
      acc += attn[:,l] * LN(v)             attn folded into LN2 affine
  - Host (fp32, ~1.5% of FLOPs, softmax-logit precision critical):
      keys/query path, softmax -> attn;  final residual z_L + gathered acc.
  - b1/b2 are zeros and ln*_g/ln*_b are ones/zeros per the problem spec;
    verified at runtime, with a full-precision host fallback if ever not.
"""

import numpy as np
import ml_dtypes

import concourse.bacc as bacc
import concourse.tile as tile
from concourse import mybir
from concourse.bass_utils import run_bass_kernel_spmd
from concourse.masks import make_identity

BF16_NP = ml_dtypes.bfloat16
BF16 = mybir.dt.bfloat16
F32 = mybir.dt.float32
AF = mybir.ActivationFunctionType
OP = mybir.AluOpType

L, D, DB = 33, 2048, 1024
B = 2048
NCORES = 8
BC = B // NCORES      # 256 batch rows per core
MB = BC // 128        # 2 b-tiles of 128 partitions
KD = D // 128         # 16 k-tiles for mm1 contraction
KB = DB // 128        # 8 k-tiles for mm2 contraction
ND1 = DB // 512       # 2 psum chunks for h
ND2 = D // 512        # 4 psum chunks for v
EPS = 1e-5

_cached_nc = None
_last_results = None


def _build():
    global _cached_nc
    if _cached_nc is not None:
        return _cached_nc

    nc = bacc.Bacc("TRN2", target_bir_lowering=False, debug=False,
                   num_devices=NCORES)
    xt = nc.dram_tensor("xt", [L, KD, 128, BC], BF16, kind="ExternalInput")
    w1 = nc.dram_tensor("w1", [L, KD, 128, DB], BF16, kind="ExternalInput")
    w2 = nc.dram_tensor("w2", [L, KB, 128, D], BF16, kind="ExternalInput")
    attn = nc.dram_tensor("attn", [MB, 128, L], F32, kind="ExternalInput")
    out = nc.dram_tensor("out", [MB, 128, D], F32, kind="ExternalOutput")

    with tile.TileContext(nc) as tc:
        with (
            tc.tile_pool(name="const", bufs=1) as cpool,
            tc.tile_pool(name="w1p", bufs=2) as w1p,
            tc.tile_pool(name="w2p", bufs=2) as w2p,
            tc.tile_pool(name="xtp", bufs=2) as xtp,
            tc.tile_pool(name="hp", bufs=2) as hp,
            tc.tile_pool(name="htp", bufs=2) as htp,
            tc.tile_pool(name="tmpp", bufs=4) as tmpp,
            tc.tile_pool(name="stats", bufs=6) as stats,
            tc.tile_pool(name="ph", bufs=2, space="PSUM") as php,
            tc.tile_pool(name="pt", bufs=2, space="PSUM") as ptp,
            tc.tile_pool(name="pv", bufs=4, space="PSUM") as pvp,
        ):
            def emit_loads(l, n_split):
                xt_sb = xtp.tile([128, KD, BC], BF16, tag="xt")
                s1 = KD // min(n_split, 4)
                for s in range(min(n_split, 4)):
                    nc.sync.dma_start(
                        out=xt_sb[:, s * s1:(s + 1) * s1, :],
                        in_=xt[l, s * s1:(s + 1) * s1].rearrange(
                            "k p b -> p k b"))
                w1_sb = w1p.tile([128, KD, DB], BF16, tag="w1")
                s2 = KD // n_split
                for s in range(n_split):
                    nc.sync.dma_start(
                        out=w1_sb[:, s * s2:(s + 1) * s2, :],
                        in_=w1[l, s * s2:(s + 1) * s2].rearrange(
                            "k p n -> p k n"))
                w2_sb = w2p.tile([128, KB, D], BF16, tag="w2")
                s3 = KB // n_split if KB >= n_split else 1
                for s in range(KB // s3):
                    nc.sync.dma_start(
                        out=w2_sb[:, s * s3:(s + 1) * s3, :],
                        in_=w2[l, s * s3:(s + 1) * s3].rearrange(
                            "k p n -> p k n"))
                return xt_sb, w1_sb, w2_sb

            # layer-0 inputs first, finely split, so the first matmul can
            # start as early as possible
            pending = emit_loads(0, 8)

            ident = cpool.tile([128, 128], BF16)
            make_identity(nc, ident)
            eps_t = cpool.tile([128, 1], F32)
            nc.vector.memset(eps_t, EPS)
            attn_sb = cpool.tile([128, MB, L], F32)
            nc.sync.dma_start(out=attn_sb, in_=attn[:].rearrange("m p l -> p m l"))
            acc = cpool.tile([128, MB, D], F32)

            for l in range(L):
                xt_sb, w1_sb, w2_sb = pending if l == 0 else emit_loads(l, 8 if l == 1 else 4)

                # ---- phase A per b-tile: mm1 + gelu + LN1 ----
                h_ln = []
                for m in range(MB):
                    msl = slice(m * 128, (m + 1) * 128)
                    phs = [php.tile([128, 512], F32, tag="ph", name=f"ph{n}")
                           for n in range(ND1)]
                    for n in range(ND1):
                        for k in range(KD):
                            nc.tensor.matmul(
                                phs[n], lhsT=xt_sb[:, k, msl],
                                rhs=w1_sb[:, k, n * 512:(n + 1) * 512],
                                start=(k == 0), stop=(k == KD - 1))
                    hg = hp.tile([128, DB], BF16, tag="hg")
                    for n in range(ND1):
                        nc.scalar.activation(
                            out=hg[:, n * 512:(n + 1) * 512], in_=phs[n],
                            func=AF.Gelu)
                    st1 = stats.tile([128, ND1, 6], F32, tag="st1")
                    for n in range(ND1):
                        nc.vector.bn_stats(
                            out=st1[:, n, :], in_=hg[:, n * 512:(n + 1) * 512])
                    mv1 = stats.tile([128, 2], F32, tag="mv1")
                    nc.vector.bn_aggr(out=mv1, in_=st1)
                    rs1 = stats.tile([128, 1], F32, tag="rs1")
                    nc.scalar.activation(out=rs1, in_=mv1[:, 1:2], func=AF.Sqrt,
                                         bias=eps_t)
                    nc.vector.reciprocal(out=rs1, in_=rs1)
                    c1 = stats.tile([128, 1], F32, tag="c1")
                    nc.vector.tensor_scalar(out=c1, in0=mv1[:, 0:1], scalar1=rs1,
                                            scalar2=-1.0, op0=OP.mult,
                                            op1=OP.mult)
                    hl = hp.tile([128, DB], BF16, tag="hl")
                    nc.scalar.activation(out=hl, in_=hg, func=AF.Identity,
                                         bias=c1, scale=rs1)
                    h_ln.append(hl)

                # ---- phase B per b-tile: transpose + mm2 + LN2 + accumulate ----
                for m in range(MB):
                    hl = h_ln[m]
                    ht = htp.tile([128, KB, 128], BF16, tag="ht")
                    for j2 in range(KB // 2):
                        pt = ptp.tile([128, 256], BF16, tag="pt")
                        for jj in range(2):
                            j = j2 * 2 + jj
                            nc.tensor.transpose(
                                pt[:, jj * 128:(jj + 1) * 128],
                                hl[:, j * 128:(j + 1) * 128], ident)
                        nc.vector.tensor_copy(
                            out=ht[:, j2 * 2:j2 * 2 + 2, :],
                            in_=pt[:].rearrange("p (j b) -> p j b", j=2))

                    st2 = stats.tile([128, ND2, 6], F32, tag="st2")
                    pvs = [pvp.tile([128, 512], F32, tag="pv", name=f"pv{n}")
                           for n in range(ND2)]
                    for n in range(ND2):
                        for k in range(KB):
                            nc.tensor.matmul(
                                pvs[n], lhsT=ht[:, k, :],
                                rhs=w2_sb[:, k, n * 512:(n + 1) * 512],
                                start=(k == 0), stop=(k == KB - 1))
                        nc.vector.bn_stats(out=st2[:, n, :], in_=pvs[n])
                    mv2 = stats.tile([128, 2], F32, tag="mv2")
                    nc.vector.bn_aggr(out=mv2, in_=st2)
                    rs2 = stats.tile([128, 1], F32, tag="rs2")
                    nc.scalar.activation(out=rs2, in_=mv2[:, 1:2], func=AF.Sqrt,
                                         bias=eps_t)
                    nc.vector.reciprocal(out=rs2, in_=rs2)
                    a2 = stats.tile([128, 1], F32, tag="a2")
                    nc.vector.tensor_mul(out=a2, in0=rs2,
                                         in1=attn_sb[:, m, l:l + 1])
                    c2 = stats.tile([128, 1], F32, tag="c2")
                    nc.vector.tensor_scalar(out=c2, in0=mv2[:, 0:1], scalar1=a2,
                                            scalar2=-1.0, op0=OP.mult,
                                            op1=OP.mult)
                    for n in range(ND2):
                        nsl = slice(n * 512, (n + 1) * 512)
                        if l == 0:
                            nc.vector.tensor_scalar(
                                out=acc[:, m, nsl], in0=pvs[n], scalar1=a2,
                                scalar2=c2, op0=OP.mult, op1=OP.add)
                        else:
                            tmp = tmpp.tile([128, 512], BF16, tag="tmp")
                            nc.scalar.activation(out=tmp, in_=pvs[n],
                                                 func=AF.Identity, bias=c2,
                                                 scale=a2)
                            nc.vector.tensor_add(out=acc[:, m, nsl],
                                                 in0=acc[:, m, nsl], in1=tmp)

            for m in range(MB):
                for q in range(4):
                    nc.sync.dma_start(
                        out=out[m, :, q * 512:(q + 1) * 512],
                        in_=acc[:, m, q * 512:(q + 1) * 512])

    nc.compile()
    _cached_nc = nc
    return nc


# ---------------- host-side math (fp32) ----------------

def _gelu(x):
    from scipy.special import erf
    return 0.5 * x * (1.0 + erf(x / np.sqrt(2.0, dtype=np.float32)))


def _ln(x, g, b):
    mu = x.mean(-1, keepdims=True, dtype=np.float32)
    var = np.square(x - mu).mean(-1, keepdims=True, dtype=np.float32)
    return (x - mu) / np.sqrt(var + EPS) * g + b


def _elu(x):
    return np.where(x > 0, x, np.expm1(np.minimum(x, 0.0)))


def _host_query_attn(zL, pos_emb, Wk, Wq1, bq1, lnq_g, lnq_b, Wq2, bq2):
    keys = pos_emb @ Wk                                   # [L, D]
    hq = _gelu(zL @ Wq1 + bq1)
    hq = _ln(hq, lnq_g, lnq_b)
    q_tr = hq @ Wq2 + bq2
    query = 1.0 + _elu(zL + q_tr)                         # [B, D]
    s = query @ keys.T                                    # [B, L]
    s -= s.max(-1, keepdims=True)
    e = np.exp(s)
    return e / e.sum(-1, keepdims=True)


def _host_reference(x, pos_emb, Wk, W1, b1, ln1_g, ln1_b, W2, b2, ln2_g,
                    ln2_b, Wq1, bq1, lnq_g, lnq_b, Wq2, bq2):
    """Full-precision fallback (only used if the affine params are ever
    non-trivial, which the problem spec's fills make impossible)."""
    zL = x[:, -1, :]
    attn = _host_query_attn(zL, pos_emb, Wk, Wq1, bq1, lnq_g, lnq_b, Wq2, bq2)
    acc = np.zeros_like(zL)
    for l in range(L):
        h = _gelu(x[:, l, :] @ W1[l] + b1[l])
        h = _ln(h, ln1_g[l], ln1_b[l])
        v = h @ W2[l] + b2[l]
        v = _ln(v, ln2_g[l], ln2_b[l])
        acc += attn[:, l:l + 1] * v
    return zL + acc


def kernel(x, pos_emb, Wk, W1, b1, ln1_g, ln1_b, W2, b2, ln2_g, ln2_b,
           Wq1, bq1, lnq_g, lnq_b, Wq2, bq2):
    global _last_results
    f32 = np.float32
    x = np.asarray(x, f32)
    pos_emb = np.asarray(pos_emb, f32)
    Wk = np.asarray(Wk, f32)
    W1 = np.asarray(W1, f32)
    b1 = np.asarray(b1, f32)
    ln1_g = np.asarray(ln1_g, f32)
    ln1_b = np.asarray(ln1_b, f32)
    W2 = np.asarray(W2, f32)
    b2 = np.asarray(b2, f32)
    ln2_g = np.asarray(ln2_g, f32)
    ln2_b = np.asarray(ln2_b, f32)
    Wq1 = np.asarray(Wq1, f32)
    bq1 = np.asarray(bq1, f32)
    lnq_g = np.asarray(lnq_g, f32)
    lnq_b = np.asarray(lnq_b, f32)
    Wq2 = np.asarray(Wq2, f32)
    bq2 = np.asarray(bq2, f32)

    trivial = (
        not b1.any() and not b2.any()
        and not ln1_b.any() and not ln2_b.any()
        and np.all(ln1_g == 1.0) and np.all(ln2_g == 1.0)
    )
    if not trivial:
        return _host_reference(x, pos_emb, Wk, W1, b1, ln1_g, ln1_b, W2, b2,
                               ln2_g, ln2_b, Wq1, bq1, lnq_g, lnq_b, Wq2, bq2)

    zL = np.ascontiguousarray(x[:, -1, :])
    attn = _host_query_attn(zL, pos_emb, Wk, Wq1, bq1, lnq_g, lnq_b, Wq2, bq2)

    w1b = np.ascontiguousarray(W1.reshape(L, KD, 128, DB)).astype(BF16_NP)
    w2b = np.ascontiguousarray(W2.reshape(L, KB, 128, D)).astype(BF16_NP)

    in_maps = []
    for c in range(NCORES):
        xsb = x[c * BC:(c + 1) * BC].astype(BF16_NP)       # [BC, L, D]
        xt = np.empty((L, KD, 128, BC), BF16_NP)
        for l in range(L):
            xt[l] = xsb[:, l, :].T.reshape(KD, 128, BC)
        attn_c = np.ascontiguousarray(
            attn[c * BC:(c + 1) * BC].reshape(MB, 128, L).astype(f32))
        in_maps.append({"xt": xt, "w1": w1b, "w2": w2b, "attn": attn_c})

    nc = _build()
    res = run_bass_kernel_spmd(nc, in_maps, list(range(NCORES)))
    _last_results = res

    parts = [r["out"].reshape(BC, D).astype(f32) for r in res.results]
    attended = np.concatenate(parts, axis=0)
    return (zL + attended).astype(f32)


# revision 18
# speedup vs baseline: 1.9676x; 1.0016x over previous
"""Trainium2 Bass kernel for nn_DWAttEncoder (depth-wise attention encoder).

Strategy (8 NeuronCores, data-parallel over batch B=2048 -> 256 rows/core):
  - Device (per core, per layer l of 33):
      h   = gelu(x[:,l,:] @ W1[l])        bf16 matmul, fp32 PSUM accum
      h   = LN(h)                          fp32 stats, fused scale/bias apply
      hT  = transpose(h)                   PE-transpose (needed as mm2 lhsT)
      v   = hT.T @ W2[l]                   bf16 matmul
      acc += attn[:,l] * LN(v)             attn folded into LN2 affine
  - Host (fp32, ~1.5% of FLOPs, softmax-logit precision critical):
      keys/query path, softmax -> attn;  final residual z_L + gathered acc.
  - b1/b2 are zeros and ln*_g/ln*_b are ones/zeros per the problem spec;
    verified at runtime, with a full-precision host fallback if ever not.
"""

import numpy as np
import ml_dtypes

import concourse.bacc as bacc
import concourse.tile as tile
from concourse import mybir
from concourse.bass_utils import run_bass_kernel_spmd
from concourse.masks import make_identity

BF16_NP = ml_dtypes.bfloat16
BF16 = mybir.dt.bfloat16
F32 = mybir.dt.float32
AF = mybir.ActivationFunctionType
OP = mybir.AluOpType

L, D, DB = 33, 2048, 1024
B = 2048
NCORES = 8
BC = B // NCORES      # 256 batch rows per core
MB = BC // 128        # 2 b-tiles of 128 partitions
KD = D // 128         # 16 k-tiles for mm1 contraction
KB = DB // 128        # 8 k-tiles for mm2 contraction
ND1 = DB // 512       # 2 psum chunks for h
ND2 = D // 512        # 4 psum chunks for v
EPS = 1e-5

_cached_nc = None
_last_results = None


def _build():
    global _cached_nc
    if _cached_nc is not None:
        return _cached_nc

    nc = bacc.Bacc("TRN2", target_bir_lowering=False, debug=False,
                   num_devices=NCORES)
    xt = nc.dram_tensor("xt", [L, KD, 128, BC], BF16, kind="ExternalInput")
    w1 = nc.dram_tensor("w1", [L, KD, 128, DB], BF16, kind="ExternalInput")
    w2 = nc.dram_tensor("w2", [L, KB, 128, D], BF16, kind="ExternalInput")
    attn = nc.dram_tensor("attn", [MB, 128, L], F32, kind="ExternalInput")
    out = nc.dram_tensor("out", [MB, 128, D], F32, kind="ExternalOutput")

    with tile.TileContext(nc) as tc:
        with (
            tc.tile_pool(name="const", bufs=1) as cpool,
            tc.tile_pool(name="w1p", bufs=2) as w1p,
            tc.tile_pool(name="w2p", bufs=2) as w2p,
            tc.tile_pool(name="xtp", bufs=2) as xtp,
            tc.tile_pool(name="hp", bufs=3) as hp,
            tc.tile_pool(name="htp", bufs=3) as htp,
            tc.tile_pool(name="tmpp", bufs=6) as tmpp,
            tc.tile_pool(name="stats", bufs=8) as stats,
            tc.tile_pool(name="ph", bufs=2, space="PSUM") as php,
            tc.tile_pool(name="pt", bufs=2, space="PSUM") as ptp,
            tc.tile_pool(name="pv", bufs=4, space="PSUM") as pvp,
        ):
            def emit_loads(l, n_split):
                xt_sb = xtp.tile([128, KD, BC], BF16, tag="xt")
                s1 = KD // min(n_split, 4)
                for s in range(min(n_split, 4)):
                    nc.sync.dma_start(
                        out=xt_sb[:, s * s1:(s + 1) * s1, :],
                        in_=xt[l, s * s1:(s + 1) * s1].rearrange(
                            "k p b -> p k b"))
                w1_sb = w1p.tile([128, KD, DB], BF16, tag="w1")
                s2 = KD // n_split
                for s in range(n_split):
                    nc.sync.dma_start(
                        out=w1_sb[:, s * s2:(s + 1) * s2, :],
                        in_=w1[l, s * s2:(s + 1) * s2].rearrange(
                            "k p n -> p k n"))
                w2_sb = w2p.tile([128, KB, D], BF16, tag="w2")
                s3 = KB // n_split if KB >= n_split else 1
                for s in range(KB // s3):
                    nc.sync.dma_start(
                        out=w2_sb[:, s * s3:(s + 1) * s3, :],
                        in_=w2[l, s * s3:(s + 1) * s3].rearrange(
                            "k p n -> p k n"))
                return xt_sb, w1_sb, w2_sb

            # layer-0 inputs first, finely split, so the first matmul can
            # start as early as possible
            pending = emit_loads(0, 8)

            ident = cpool.tile([128, 128], BF16)
            make_identity(nc, ident)
            eps_t = cpool.tile([128, 1], F32)
            nc.vector.memset(eps_t, EPS)
            attn_sb = cpool.tile([128, MB, L], F32)
            nc.sync.dma_start(out=attn_sb, in_=attn[:].rearrange("m p l -> p m l"))
            acc = cpool.tile([128, MB, D], F32)

            for l in range(L):
                xt_sb, w1_sb, w2_sb = pending if l == 0 else emit_loads(l, 8 if l == 1 else 4)

                # ---- phase A per b-tile: mm1 + gelu + LN1 ----
                h_ln = []
                for m in range(MB):
                    msl = slice(m * 128, (m + 1) * 128)
                    phs = [php.tile([128, 512], F32, tag="ph", name=f"ph{n}")
                           for n in range(ND1)]
                    for n in range(ND1):
                        for k in range(KD):
                            nc.tensor.matmul(
                                phs[n], lhsT=xt_sb[:, k, msl],
                                rhs=w1_sb[:, k, n * 512:(n + 1) * 512],
                                start=(k == 0), stop=(k == KD - 1))
                    hg = hp.tile([128, DB], BF16, tag="hg")
                    for n in range(ND1):
                        nc.scalar.activation(
                            out=hg[:, n * 512:(n + 1) * 512], in_=phs[n],
                            func=AF.Gelu)
                    st1 = stats.tile([128, ND1, 6], F32, tag="st1")
                    for n in range(ND1):
                        nc.vector.bn_stats(
                            out=st1[:, n, :], in_=hg[:, n * 512:(n + 1) * 512])
                    mv1 = stats.tile([128, 2], F32, tag="mv1")
                    nc.vector.bn_aggr(out=mv1, in_=st1)
                    rs1 = stats.tile([128, 1], F32, tag="rs1")
                    nc.scalar.activation(out=rs1, in_=mv1[:, 1:2], func=AF.Sqrt,
                                         bias=eps_t)
                    nc.vector.reciprocal(out=rs1, in_=rs1)
                    c1 = stats.tile([128, 1], F32, tag="c1")
                    nc.vector.tensor_scalar(out=c1, in0=mv1[:, 0:1], scalar1=rs1,
                                            scalar2=-1.0, op0=OP.mult,
                                            op1=OP.mult)
                    hl = hp.tile([128, DB], BF16, tag="hl")
                    nc.scalar.activation(out=hl, in_=hg, func=AF.Identity,
                                         bias=c1, scale=rs1)
                    h_ln.append(hl)

                # ---- phase B per b-tile: transpose + mm2 + LN2 + accumulate ----
                for m in range(MB):
                    hl = h_ln[m]
                    ht = htp.tile([128, KB, 128], BF16, tag="ht")
                    for j2 in range(KB // 2):
                        pt = ptp.tile([128, 256], BF16, tag="pt")
                        for jj in range(2):
                            j = j2 * 2 + jj
                            nc.tensor.transpose(
                                pt[:, jj * 128:(jj + 1) * 128],
                                hl[:, j * 128:(j + 1) * 128], ident)
                        nc.vector.tensor_copy(
                            out=ht[:, j2 * 2:j2 * 2 + 2, :],
                            in_=pt[:].rearrange("p (j b) -> p j b", j=2))

                    st2 = stats.tile([128, ND2, 6], F32, tag="st2")
                    pvs = [pvp.tile([128, 512], F32, tag="pv", name=f"pv{n}")
                           for n in range(ND2)]
                    for n in range(ND2):
                        for k in range(KB):
                            nc.tensor.matmul(
                                pvs[n], lhsT=ht[:, k, :],
                                rhs=w2_sb[:, k, n * 512:(n + 1) * 512],
                                start=(k == 0), stop=(k == KB - 1))
                        nc.vector.bn_stats(out=st2[:, n, :], in_=pvs[n])
                    mv2 = stats.tile([128, 2], F32, tag="mv2")
                    nc.vector.bn_aggr(out=mv2, in_=st2)
                    rs2 = stats.tile([128, 1], F32, tag="rs2")
                    nc.scalar.activation(out=rs2, in_=mv2[:, 1:2], func=AF.Sqrt,
                                         bias=eps_t)
                    nc.vector.reciprocal(out=rs2, in_=rs2)
                    a2 = stats.tile([128, 1], F32, tag="a2")
                    nc.vector.tensor_mul(out=a2, in0=rs2,
                                         in1=attn_sb[:, m, l:l + 1])
                    c2 = stats.tile([128, 1], F32, tag="c2")
                    nc.vector.tensor_scalar(out=c2, in0=mv2[:, 0:1], scalar1=a2,
                                            scalar2=-1.0, op0=OP.mult,
                                            op1=OP.mult)
                    for n in range(ND2):
                        nsl = slice(n * 512, (n + 1) * 512)
                        if l == 0:
                            nc.vector.tensor_scalar(
                                out=acc[:, m, nsl], in0=pvs[n], scalar1=a2,
                                scalar2=c2, op0=OP.mult, op1=OP.add)
                        else:
                            tmp = tmpp.tile([128, 512], BF16, tag="tmp")
                            nc.scalar.activation(out=tmp, in_=pvs[n],
                                                 func=AF.Identity, bias=c2,
                                                 scale=a2)
                            nc.vector.tensor_add(out=acc[:, m, nsl],
                                                 in0=acc[:, m, nsl], in1=tmp)

            for m in range(MB):
                for q in range(4):
                    nc.sync.dma_start(
                        out=out[m, :, q * 512:(q + 1) * 512],
                        in_=acc[:, m, q * 512:(q + 1) * 512])

    nc.compile()
    _cached_nc = nc
    return nc


# ---------------- host-side math (fp32) ----------------

def _gelu(x):
    from scipy.special import erf
    return 0.5 * x * (1.0 + erf(x / np.sqrt(2.0, dtype=np.float32)))


def _ln(x, g, b):
    mu = x.mean(-1, keepdims=True, dtype=np.float32)
    var = np.square(x - mu).mean(-1, keepdims=True, dtype=np.float32)
    return (x - mu) / np.sqrt(var + EPS) * g + b


def _elu(x):
    return np.where(x > 0, x, np.expm1(np.minimum(x, 0.0)))


def _host_query_attn(zL, pos_emb, Wk, Wq1, bq1, lnq_g, lnq_b, Wq2, bq2):
    keys = pos_emb @ Wk                                   # [L, D]
    hq = _gelu(zL @ Wq1 + bq1)
    hq = _ln(hq, lnq_g, lnq_b)
    q_tr = hq @ Wq2 + bq2
    query = 1.0 + _elu(zL + q_tr)                         # [B, D]
    s = query @ keys.T                                    # [B, L]
    s -= s.max(-1, keepdims=True)
    e = np.exp(s)
    return e / e.sum(-1, keepdims=True)


def _host_reference(x, pos_emb, Wk, W1, b1, ln1_g, ln1_b, W2, b2, ln2_g,
                    ln2_b, Wq1, bq1, lnq_g, lnq_b, Wq2, bq2):
    """Full-precision fallback (only used if the affine params are ever
    non-trivial, which the problem spec's fills make impossible)."""
    zL = x[:, -1, :]
    attn = _host_query_attn(zL, pos_emb, Wk, Wq1, bq1, lnq_g, lnq_b, Wq2, bq2)
    acc = np.zeros_like(zL)
    for l in range(L):
        h = _gelu(x[:, l, :] @ W1[l] + b1[l])
        h = _ln(h, ln1_g[l], ln1_b[l])
        v = h @ W2[l] + b2[l]
        v = _ln(v, ln2_g[l], ln2_b[l])
        acc += attn[:, l:l + 1] * v
    return zL + acc


def kernel(x, pos_emb, Wk, W1, b1, ln1_g, ln1_b, W2, b2, ln2_g, ln2_b,
           Wq1, bq1, lnq_g, lnq_b, Wq2, bq2):
    global _last_results
    f32 = np.float32
    x = np.asarray(x, f32)
    pos_emb = np.asarray(pos_emb, f32)
    Wk = np.asarray(Wk, f32)
    W1 = np.asarray(W1, f32)
    b1 = np.asarray(b1, f32)
    ln1_g = np.asarray(ln1_g, f32)
    ln1_b = np.asarray(ln1_b, f32)
    W2 = np.asarray(W2, f32)
    b2 = np.asarray(b2, f32)
    ln2_g = np.asarray(ln2_g, f32)
    ln2_b = np.asarray(ln2_b, f32)
    Wq1 = np.asarray(Wq1, f32)
    bq1 = np.asarray(bq1, f32)
    lnq_g = np.asarray(lnq_g, f32)
    lnq_b = np.asarray(lnq_b, f32)
    Wq2 = np.asarray(Wq2, f32)
    bq2 = np.asarray(bq2, f32)

    trivial = (
        not b1.any() and not b2.any()
        and not ln1_b.any() and not ln2_b.any()
        and np.all(ln1_g == 1.0) and np.all(ln2_g == 1.0)
    )
    if not trivial:
        return _host_reference(x, pos_emb, Wk, W1, b1, ln1_g, ln1_b, W2, b2,
                               ln2_g, ln2_b, Wq1, bq1, lnq_g, lnq_b, Wq2, bq2)

    zL = np.ascontiguousarray(x[:, -1, :])
    attn = _host_query_attn(zL, pos_emb, Wk, Wq1, bq1, lnq_g, lnq_b, Wq2, bq2)

    w1b = np.ascontiguousarray(W1.reshape(L, KD, 128, DB)).astype(BF16_NP)
    w2b = np.ascontiguousarray(W2.reshape(L, KB, 128, D)).astype(BF16_NP)

    in_maps = []
    for c in range(NCORES):
        xsb = x[c * BC:(c + 1) * BC].astype(BF16_NP)       # [BC, L, D]
        xt = np.empty((L, KD, 128, BC), BF16_NP)
        for l in range(L):
            xt[l] = xsb[:, l, :].T.reshape(KD, 128, BC)
        attn_c = np.ascontiguousarray(
            attn[c * BC:(c + 1) * BC].reshape(MB, 128, L).astype(f32))
        in_maps.append({"xt": xt, "w1": w1b, "w2": w2b, "attn": attn_c})

    nc = _build()
    res = run_bass_kernel_spmd(nc, in_maps, list(range(NCORES)))
    _last_results = res

    parts = [r["out"].reshape(BC, D).astype(f32) for r in res.results]
    attended = np.concatenate(parts, axis=0)
    return (zL + attended).astype(f32)
